# revision 3
# baseline (speedup 1.0000x reference)
"""AxialRoPE self-attention on 8 Trainium2 NeuronCores.

Sharding: 8 cores = 4 batches x 2 head-groups (8 heads each).
Each core computes q/k/v projections for its head-group over the full
sequence of its batch, RoPE, attention, and a partial output projection
(row-sharded Wo). Host sums the two partial outputs per batch.

Per-core kernel (all matmuls bf16 with fp32 PSUM accumulation):
  x [2048, 1024] bf16 natural layout; device transposes via XBAR DMA
  into xt [128, S] tiles (no host-side transpose).
  QT = Wq^T x^T + bq   [512, 2048] head-dim-major; same for K; V natural.
  RoPE (natural head-dim rows): psw = SWP @ QT swaps partition pairs
  (2i <-> 2i+1) on the PE; qt' = QT*cosT + psw*sinTs where sinTs carries
  the (-1)^(d+1) sign.
  scoresT[ks, qs] per head, 2 heads packed in the PE array (K=64 row tiles),
  both heads' scores in one [128, 2048] psum tile -> single exp (scalar
  engine, scale=1/8, no max subtraction: scores are ~N(0,1), max < 7).
  PV: lhsT = V_aug [ks, 65] (65th column of ones -> row 64 = softmax
  denominator). Normalization: recip = exp(-ln(sum)) on ACT, broadcast via
  a K=1 ones matmul, applied on the DVE.
"""

import os
from concurrent.futures import ThreadPoolExecutor

import numpy as np

B, S, D = 4, 2048, 1024
NHEAD, HDIM = 16, 64
HG = 2                # head-group shards
HPC = NHEAD // HG     # 8 heads per core
DG = HPC * HDIM       # 512 local projection width
NCORES = 8
ROPE_BASE = 10000.0

_CACHE = {}
_POOL = ThreadPoolExecutor(max_workers=8)


def _build_program():
    from concourse import bass, bacc, tile
    from concourse import mybir

    dt = mybir.dt
    f32, bf16 = dt.float32, dt.bfloat16
    AF = mybir.ActivationFunctionType
    ALU = mybir.AluOpType
    PSUM = bass.MemorySpace.PSUM

    nc = bacc.Bacc("TRN2", target_bir_lowering=False, debug=False)

    # The PJRT-side NEFF cache keys on the HLO signature, which sees only
    # tensor shapes -- encode a build nonce in a dummy input's shape so
    # program variants with identical I/O still recompile.
    _nw = (int(os.environ.get("BUILD_REPEAT", "1"))
           + 100 * int(os.environ.get("BUILD_NONCE", "0")))
    nc.dram_tensor("nonce", [1, _nw], f32, kind="ExternalInput")

    x_d = nc.dram_tensor("x", [S, D], bf16, kind="ExternalInput")
    wq_d = nc.dram_tensor("wq", [D, DG], bf16, kind="ExternalInput")
    wk_d = nc.dram_tensor("wk", [D, DG], bf16, kind="ExternalInput")
    wv_d = nc.dram_tensor("wv", [D, DG], bf16, kind="ExternalInput")
    wo_d = nc.dram_tensor("wo", [DG, D], bf16, kind="ExternalInput")
    cos_d = nc.dram_tensor("cosT", [128, S], bf16, kind="ExternalInput")
    sin_d = nc.dram_tensor("sinTs", [128, S], bf16, kind="ExternalInput")
    swp_d = nc.dram_tensor("swp", [128, 128], bf16, kind="ExternalInput")
    bq_d = nc.dram_tensor("bq4", [128, 4], f32, kind="ExternalInput")
    bk_d = nc.dram_tensor("bk4", [128, 4], f32, kind="ExternalInput")
    bv_d = nc.dram_tensor("bv", [1, DG], bf16, kind="ExternalInput")
    bo_d = nc.dram_tensor("bo", [1, D], bf16, kind="ExternalInput")
    out_d = nc.dram_tensor("out", [S, D], f32, kind="ExternalOutput")

    CC = D // 128    # 8 contraction chunks
    DC = DG // 128   # 4 dout chunks (2 heads each)
    SC = S // 128    # 16 sequence chunks
    QT2 = 512        # query tile
    NQ = S // QT2    # 4
    NPP = NQ * 2     # denominator points per pair

    with tile.TileContext(nc) as tc:
        with (
            tc.tile_pool(name="persist", bufs=1) as P,
            tc.tile_pool(name="ps_all", bufs=2, space=PSUM) as PS,
            tc.tile_pool(name="tmp", bufs=2) as T1,
            tc.tile_pool(name="wstream", bufs=16) as WS,
            tc.tile_pool(name="wvp", bufs=8) as WV,
            tc.tile_pool(name="ptp", bufs=2) as PT,
            tc.tile_pool(name="nrm", bufs=4) as NR,
        ):
            _REPEAT = int(os.environ.get("BUILD_REPEAT", "1"))
            for _rep in range(_REPEAT):
                ones = P.tile([1, 128], bf16, tag="ones")
                nc.vector.memset(ones[:], 1.0)
                ones65 = P.tile([65, 64], f32, tag="ones65")
                nc.vector.memset(ones65[:], 1.0)
                qt = [P.tile([128, S], bf16, tag=f"qt{i}", name=f"qt{i}") for i in range(DC)]
                kt = [P.tile([128, S], bf16, tag=f"kt{i}", name=f"kt{i}") for i in range(DC)]
                vaug = [P.tile([128, HPC * 65], bf16, tag=f"va{i}", name=f"va{i}") for i in range(SC)]
                aoh = [P.tile([64, S], bf16, tag=f"aoh{i}", name=f"aoh{i}") for i in range(HPC)]
                xt = [P.tile([128, S], bf16, tag=f"xt{i}", name=f"xt{i}") for i in range(CC)]
                for i in range(CC):
                    nc.sync.dma_start_transpose(
                        xt[i][:], x_d.ap()[:, i * 128:(i + 1) * 128]
                    )
                cos_t = P.tile([128, S], bf16, tag="cos")
                sin_t = P.tile([128, S], bf16, tag="sin")
                nc.sync.dma_start(cos_t[:], cos_d.ap()[:])
                nc.sync.dma_start(sin_t[:], sin_d.ap()[:])
                swp_t = P.tile([128, 128], bf16, tag="swp")
                nc.sync.dma_start(swp_t[:], swp_d.ap()[:])
                bq4 = P.tile([128, 4], f32, tag="bq4")
                bk4 = P.tile([128, 4], f32, tag="bk4")
                bv_sb = P.tile([1, DG], bf16, tag="bv_sb")
                nc.sync.dma_start(bq4[:], bq_d.ap()[:])
                nc.sync.dma_start(bk4[:], bk_d.ap()[:])
                nc.sync.dma_start(bv_sb[:], bv_d.ap()[:])

                # ---- per-chunk: Q/K projection + rope, then attention ----
                wqk_sb = {}
                for wi, w_d in enumerate([wq_d, wk_d]):
                    wqk_sb[wi] = [WS.tile([128, DG], bf16, tag="w", name=f"w{wi}_{_}") for _ in range(CC)]
                    for i in range(CC):
                        nc.sync.dma_start(wqk_sb[wi][i][:], w_d.ap()[i * 128:(i + 1) * 128, :])
                def emit_qk(dc):
                    dsl = slice(dc * 128, (dc + 1) * 128)
                    for wi, (b4, dst) in enumerate([(bq4, qt), (bk4, kt)]):
                        w_sb = wqk_sb[wi]
                        qtsb = T1.tile([128, S], bf16, tag="qtsb", bufs=2)
                        tt2 = T1.tile([128, S], bf16, tag="tt2", bufs=1)
                        for st in range(4):
                            sl = slice(st * 512, (st + 1) * 512)
                            ps = PS.tile([128, 512], f32, tag="proj", name="psp")
                            for cc in range(CC):
                                nc.tensor.matmul(
                                    ps[:], w_sb[cc][:, dsl], xt[cc][:, sl],
                                    start=(cc == 0), stop=(cc == CC - 1),
                                )
                            nc.vector.tensor_scalar(
                                qtsb[:, sl], ps[:], b4[:, dc:dc + 1], None,
                                op0=ALU.add,
                            )
                        # rotate-half partners via PE pair-swap matmuls; a
                        # second pass so the swap of chunk 0 never stalls
                        # the PE behind chunk 0's bias-add on the DVE.
                        for st in range(4):
                            sl = slice(st * 512, (st + 1) * 512)
                            psw = PS.tile([128, 512], f32, tag="proj", name="psw")
                            nc.tensor.matmul(
                                psw[:], swp_t[:], qtsb[:, sl],
                                start=True, stop=True,
                            )
                            nc.vector.tensor_tensor(
                                dst[dc][:, sl], qtsb[:, sl], cos_t[:, sl],
                                op=ALU.mult,
                            )
                            nc.vector.tensor_tensor(
                                tt2[:, sl], psw[:], sin_t[:, sl], op=ALU.mult
                            )
                            nc.vector.tensor_tensor(
                                dst[dc][:, sl], dst[dc][:, sl], tt2[:, sl],
                                op=ALU.add,
                            )

                def emit_attn(dc):
                    # ---- attention for head pair dc ----
                    pr = dc
                    rscoll = NR.tile([65, NPP * QT2], f32, tag="rscoll", bufs=1, name="rscoll")
                    for q in range(NQ):
                        qsl = slice(q * QT2, (q + 1) * QT2)
                        pso = [
                            PS.tile([65, QT2], f32, tag="psoA", name="psoA", bufs=1),
                            PS.tile([65, QT2], f32, tag="psoB", name="psoB", bufs=1),
                        ]
                        for ks in range(SC):
                            ksl = slice(ks * 128, (ks + 1) * 128)
                            pss = PS.tile([128, 2 * QT2], f32, tag="big", name="pss")
                            for half in range(2):
                                rows = slice(64 * half, 64 * half + 64)
                                nc.tensor.matmul(
                                    pss[:, half * QT2:(half + 1) * QT2],
                                    kt[pr][rows, ksl],
                                    qt[pr][rows, qsl],
                                    start=True, stop=True,
                                )
                            ptile = PT.tile([128, 2 * QT2], bf16, tag="pt", name="ptile", bufs=3)
                            nc.scalar.activation(ptile[:], pss[:], AF.Exp, scale=0.125)
                            for half in range(2):
                                lh = 2 * pr + half
                                nc.tensor.matmul(
                                    pso[half][:],
                                    vaug[ks][:, 65 * lh:65 * lh + 65],
                                    ptile[:, half * QT2:(half + 1) * QT2],
                                    start=(ks == 0), stop=(ks == SC - 1),
                                )
                        for half in range(2):
                            lh = 2 * pr + half
                            csl = slice((q * 2 + half) * QT2, (q * 2 + half + 1) * QT2)
                            nc.vector.tensor_copy(aoh[lh][:, qsl], pso[half][0:64, :])
                            nc.vector.tensor_copy(rscoll[64:65, csl], pso[half][64:65, :])
                    # batched denominators for this pair: one Ln + one Exp,
                    # then per-point broadcast and in-place normalize of aoh
                    nc.scalar.activation(rscoll[64:65, :], rscoll[64:65, :], AF.Ln)
                    nc.scalar.activation(
                        rscoll[64:65, :], rscoll[64:65, :], AF.Exp, scale=-1.0
                    )
                    for q in range(NQ):
                        qsl = slice(q * QT2, (q + 1) * QT2)
                        for half in range(2):
                            lh = 2 * pr + half
                            csl = slice((q * 2 + half) * QT2, (q * 2 + half + 1) * QT2)
                            psb = PS.tile([64, QT2], f32, tag="psoA", name="psb", bufs=1)
                            nc.tensor.matmul(
                                psb[:], ones65[64:65, 0:64],
                                rscoll[64:65, csl], start=True, stop=True,
                            )
                            recb = NR.tile([64, QT2], f32, tag="recb", bufs=1)
                            nc.vector.tensor_copy(recb[:], psb[:])
                            nc.vector.tensor_tensor(
                                aoh[lh][:, qsl], aoh[lh][:, qsl], recb[:],
                                op=ALU.mult,
                            )

                emit_qk(0)
                # ---- V projection first (attention needs all of it) ----
                wv_sb = [WV.tile([128, DG], bf16, tag="wv", name=f"wv_{_}") for _ in range(CC)]
                for i in range(CC):
                    nc.sync.dma_start(wv_sb[i][:], wv_d.ap()[i * 128:(i + 1) * 128, :])
                for sc in range(SC):
                    ssl = slice(sc * 128, (sc + 1) * 128)
                    ps = PS.tile([128, 512], f32, tag="proj", name="psv")
                    for cc in range(CC):
                        nc.tensor.matmul(
                            ps[:], xt[cc][:, ssl], wv_sb[cc][:],
                            start=(cc == 0), stop=False,
                        )
                    nc.tensor.matmul(
                        ps[:], ones[0:1, 0:128], bv_sb[:], start=False, stop=True,
                    )
                    va3 = vaug[sc][:].rearrange("p (h c) -> p h c", c=65)
                    ps3 = ps[:].rearrange("p (h c) -> p h c", c=64)
                    nc.vector.tensor_copy(va3[:, :, 0:64], ps3[:, :, :])
                    nc.vector.memset(va3[:, :, 64:65], 1.0)

                emit_attn(0)
                for dc in range(1, DC):
                    emit_qk(dc)
                    emit_attn(dc)

                # ---- output projection ----
                wo_sb = [P.tile([64, D], bf16, tag=f"wo{i}", name=f"wo{i}") for i in range(HPC)]
                for i in range(HPC):
                    nc.sync.dma_start(wo_sb[i][:], wo_d.ap()[i * 64:(i + 1) * 64, :])
                bo_sb = P.tile([1, D], bf16, tag="bo")
                nc.sync.dma_start(bo_sb[:], bo_d.ap()[:])
                for sc in range(SC):
                    ssl = slice(sc * 128, (sc + 1) * 128)
                    ps = PS.tile([128, 2 * QT2], f32, tag="big", name="pso3")
                    for nt in range(2):
                        nsl = slice(nt * 512, (nt + 1) * 512)
                        for h8 in range(HPC):
                            nc.tensor.matmul(
                                ps[:, nsl], aoh[h8][:, ssl], wo_sb[h8][:, nsl],
                                start=(h8 == 0), stop=False,
                            )
                        nc.tensor.matmul(
                            ps[:, nsl], ones[0:1, 0:128], bo_sb[0:1, nsl],
                            start=False, stop=True,
                        )
                    ob = T1.tile([128, D], f32, tag="qtsb", name="ob", bufs=2)
                    nc.vector.tensor_copy(ob[:], ps[:])
                    nc.sync.dma_start(out_d.ap()[ssl, :], ob[:])

    nc.compile()
    return nc


def _rope_tables(start):
    """Natural-layout rope tables: row p pairs (2f, 2f+1); sinTs carries
    the rotate-half sign (-1 on even rows, +1 on odd)."""
    inv_freq = (1.0 / (ROPE_BASE ** (np.arange(0, HDIM, 2, dtype=np.float64)
                                     / HDIM))).astype(np.float32)
    pos = np.arange(S, dtype=np.float32)
    rel = np.maximum(pos - start, 0.0)
    ang = inv_freq[:, None] * rel[None, :]          # [32, S]
    c64 = np.repeat(np.cos(ang), 2, axis=0)         # [64, S]
    s64 = np.repeat(np.sin(ang), 2, axis=0)
    sign = np.where(np.arange(HDIM) % 2 == 0, -1.0, 1.0).astype(np.float32)
    s64 = s64 * sign[:, None]
    cosT = np.concatenate([c64, c64], axis=0)       # [128, S]
    sinTs = np.concatenate([s64, s64], axis=0)
    return cosT, sinTs


def _swap_matrix():
    """SWP[k, m] = 1 iff m == k^1: psw = SWP^T-matmul gives psw[p]=q[p^1]."""
    swp = np.zeros((128, 128), dtype=np.float32)
    idx = np.arange(128)
    swp[idx, idx ^ 1] = 1.0
    return swp


def prepare_in_maps(inputs):
    import ml_dtypes

    bf16 = ml_dtypes.bfloat16
    x = np.asarray(inputs["x"])
    start = int(np.asarray(inputs["rope_start_index"]))

    jobs = {}
    for b in range(B):
        jobs[f"x{b}"] = _POOL.submit(
            lambda b=b: np.asarray(x[b], dtype=np.float32).astype(bf16))

    def prep_w(hg):
        csl = slice(hg * DG, (hg + 1) * DG)
        m = {}
        for name in ("q", "k", "v"):
            m["w" + name] = np.asarray(
                inputs["W" + name], dtype=np.float32)[:, csl].astype(bf16)
        m["wo"] = np.asarray(
            inputs["Wo"], dtype=np.float32)[csl, :].astype(bf16)
        for name in ("q", "k"):
            bvec = np.asarray(inputs["b" + name], dtype=np.float32)[csl]
            m["b" + name + "4"] = np.ascontiguousarray(
                bvec.reshape(4, 128).T).astype(np.float32)
        m["bv"] = np.asarray(inputs["bv"], dtype=np.float32)[None, csl].astype(bf16)
        bo = np.asarray(inputs["bo"], dtype=np.float32)
        m["bo"] = (bo if hg == 0 else np.zeros_like(bo))[None, :].astype(bf16)
        return m

    for hg in range(HG):
        jobs[f"w{hg}"] = _POOL.submit(prep_w, hg)
    jobs["tab"] = _POOL.submit(_rope_tables, start)

    cosT, sinTs = jobs["tab"].result()
    cosT = cosT.astype(bf16)
    sinTs = sinTs.astype(bf16)
    swp = _swap_matrix().astype(bf16)
    xbs = [jobs[f"x{b}"].result() for b in range(B)]
    per_hg = [jobs[f"w{hg}"].result() for hg in range(HG)]

    _nw = (int(os.environ.get("BUILD_REPEAT", "1"))
           + 100 * int(os.environ.get("BUILD_NONCE", "0")))
    nonce = np.zeros((1, _nw), np.float32)
    in_maps = []
    for c in range(NCORES):
        b, hg = c // HG, c % HG
        m = per_hg[hg]
        in_maps.append({
            "nonce": nonce,
            "x": xbs[b],
            "wq": m["wq"], "wk": m["wk"], "wv": m["wv"], "wo": m["wo"],
            "cosT": cosT, "sinTs": sinTs, "swp": swp,
            "bq4": m["bq4"], "bk4": m["bk4"],
            "bv": m["bv"], "bo": m["bo"],
        })
    return in_maps


def kernel(**inputs):
    from concourse.bass_utils import run_bass_kernel_spmd

    if "nc" not in _CACHE:
        _CACHE["nc"] = _build_program()
    nc = _CACHE["nc"]

    in_maps = prepare_in_maps(inputs)
    res = run_bass_kernel_spmd(nc, in_maps, core_ids=list(range(NCORES)))
    out = np.empty((B, S, D), dtype=np.float32)

    def assemble(b):
        np.add(res.results[HG * b]["out"], res.results[HG * b + 1]["out"],
               out=out[b])
    list(_POOL.map(assemble, range(B)))
    return out


# revision 7
# speedup vs baseline: 3.4089x; 3.4089x over previous
"""AxialRoPE self-attention on 8 Trainium2 NeuronCores.

Sharding: 8 cores = 4 batches x 2 head-groups (8 heads each).
Each core computes q/k/v projections for its head-group over the full
sequence of its batch, RoPE, attention, and a partial output projection
(row-sharded Wo). Host sums the two partial outputs per batch.

Per-core kernel (all matmuls bf16 with fp32 PSUM accumulation):
  x [2048, 1024] bf16 natural layout; device transposes via XBAR DMA
  into xt [128, S] tiles (no host-side transpose).
  QT = Wq^T x^T + bq   [512, 2048] head-dim-major; same for K; V natural.
  RoPE (natural head-dim rows): psw = SWP @ QT swaps partition pairs
  (2i <-> 2i+1) on the PE; qt' = QT*cosT + psw*sinTs where sinTs carries
  the (-1)^(d+1) sign.
  scoresT[ks, qs] per head, 2 heads packed in the PE array (K=64 row tiles),
  both heads' scores in one [128, 2048] psum tile -> single exp (scalar
  engine, scale=1/8, no max subtraction: scores are ~N(0,1), max < 7).
  PV: lhsT = V_aug [ks, 65] (65th column of ones -> row 64 = softmax
  denominator). Normalization: recip = exp(-ln(sum)) on ACT, broadcast via
  a K=1 ones matmul, applied on the DVE.
"""

import os
from concurrent.futures import ThreadPoolExecutor

import numpy as np

B, S, D = 4, 2048, 1024
NHEAD, HDIM = 16, 64
HG = 2                # head-group shards
HPC = NHEAD // HG     # 8 heads per core
DG = HPC * HDIM       # 512 local projection width
NCORES = 8
ROPE_BASE = 10000.0

_CACHE = {}
_POOL = ThreadPoolExecutor(max_workers=8)


def _build_program():
    from concourse import bass, bacc, tile
    from concourse import mybir

    dt = mybir.dt
    f32, bf16 = dt.float32, dt.bfloat16
    AF = mybir.ActivationFunctionType
    ALU = mybir.AluOpType
    PSUM = bass.MemorySpace.PSUM

    nc = bacc.Bacc("TRN2", target_bir_lowering=False, debug=False)

    # The PJRT-side NEFF cache keys on the HLO signature, which sees only
    # tensor shapes -- encode a build nonce in a dummy input's shape so
    # program variants with identical I/O still recompile.
    _nw = (int(os.environ.get("BUILD_REPEAT", "1"))
           + 100 * int(os.environ.get("BUILD_NONCE", "0")))
    nc.dram_tensor("nonce", [1, _nw], f32, kind="ExternalInput")

    x_d = nc.dram_tensor("x", [S, D], bf16, kind="ExternalInput")
    wq_d = nc.dram_tensor("wq", [D, DG], bf16, kind="ExternalInput")
    wk_d = nc.dram_tensor("wk", [D, DG], bf16, kind="ExternalInput")
    wv_d = nc.dram_tensor("wv", [D, DG], bf16, kind="ExternalInput")
    wo_d = nc.dram_tensor("wo", [DG, D], bf16, kind="ExternalInput")
    cos_d = nc.dram_tensor("cosT", [128, S], bf16, kind="ExternalInput")
    sin_d = nc.dram_tensor("sinTs", [128, S], bf16, kind="ExternalInput")
    swp_d = nc.dram_tensor("swp", [128, 128], bf16, kind="ExternalInput")
    bq_d = nc.dram_tensor("bq4", [128, 4], f32, kind="ExternalInput")
    bk_d = nc.dram_tensor("bk4", [128, 4], f32, kind="ExternalInput")
    bv_d = nc.dram_tensor("bv", [1, DG], bf16, kind="ExternalInput")
    bo_d = nc.dram_tensor("bo", [1, D], bf16, kind="ExternalInput")
    out_d = nc.dram_tensor("out", [S, D], f32, kind="ExternalOutput")

    CC = D // 128    # 8 contraction chunks
    DC = DG // 128   # 4 dout chunks (2 heads each)
    SC = S // 128    # 16 sequence chunks
    QT2 = 512        # query tile
    NQ = S // QT2    # 4
    NPP = NQ * 2     # denominator points per pair

    with tile.TileContext(nc) as tc:
        with (
            tc.tile_pool(name="persist", bufs=1) as P,
            tc.tile_pool(name="ps_all", bufs=2, space=PSUM) as PS,
            tc.tile_pool(name="tmp", bufs=2) as T1,
            tc.tile_pool(name="wstream", bufs=16) as WS,
            tc.tile_pool(name="wvp", bufs=8) as WV,
            tc.tile_pool(name="ptp", bufs=2) as PT,
            tc.tile_pool(name="nrm", bufs=4) as NR,
        ):
            _REPEAT = int(os.environ.get("BUILD_REPEAT", "1"))
            for _rep in range(_REPEAT):
                ones = P.tile([1, 128], bf16, tag="ones")
                nc.vector.memset(ones[:], 1.0)
                ones65 = P.tile([65, 64], f32, tag="ones65")
                nc.vector.memset(ones65[:], 1.0)
                qt = [P.tile([128, S], bf16, tag=f"qt{i}", name=f"qt{i}") for i in range(DC)]
                kt = [P.tile([128, S], bf16, tag=f"kt{i}", name=f"kt{i}") for i in range(DC)]
                vaug = [P.tile([128, HPC * 65], bf16, tag=f"va{i}", name=f"va{i}") for i in range(SC)]
                # head-pair j: partitions 0-63 = head 2j, 64-127 = head 2j+1,
                # so the output projection contracts K=128 per pair.
                aoh2 = [P.tile([128, S], bf16, tag=f"ao{i}", name=f"ao{i}") for i in range(DC)]
                xt = [P.tile([128, S], bf16, tag=f"xt{i}", name=f"xt{i}") for i in range(CC)]
                for i in range(CC):
                    nc.sync.dma_start_transpose(
                        xt[i][:], x_d.ap()[:, i * 128:(i + 1) * 128]
                    )
                cos_t = P.tile([128, S], bf16, tag="cos")
                sin_t = P.tile([128, S], bf16, tag="sin")
                nc.sync.dma_start(cos_t[:], cos_d.ap()[:])
                nc.sync.dma_start(sin_t[:], sin_d.ap()[:])
                swp_t = P.tile([128, 128], bf16, tag="swp")
                nc.sync.dma_start(swp_t[:], swp_d.ap()[:])
                bq4 = P.tile([128, 4], f32, tag="bq4")
                bk4 = P.tile([128, 4], f32, tag="bk4")
                bv_sb = P.tile([1, DG], bf16, tag="bv_sb")
                nc.sync.dma_start(bq4[:], bq_d.ap()[:])
                nc.sync.dma_start(bk4[:], bk_d.ap()[:])
                nc.sync.dma_start(bv_sb[:], bv_d.ap()[:])

                # ---- per-chunk: Q/K projection + rope, then attention ----
                wqk_sb = {}
                for wi, w_d in enumerate([wq_d, wk_d]):
                    wqk_sb[wi] = [WS.tile([128, DG], bf16, tag="w", name=f"w{wi}_{_}") for _ in range(CC)]
                    for i in range(CC):
                        nc.sync.dma_start(wqk_sb[wi][i][:], w_d.ap()[i * 128:(i + 1) * 128, :])
                def emit_qk(dc):
                    dsl = slice(dc * 128, (dc + 1) * 128)
                    for wi, (b4, dst) in enumerate([(bq4, qt), (bk4, kt)]):
                        w_sb = wqk_sb[wi]
                        qtsb = T1.tile([128, S], bf16, tag="qtsb", bufs=2)
                        tt2 = T1.tile([128, S], bf16, tag="tt2", bufs=1)
                        for st in range(4):
                            sl = slice(st * 512, (st + 1) * 512)
                            ps = PS.tile([128, 512], f32, tag="proj", name="psp")
                            for cc in range(CC):
                                nc.tensor.matmul(
                                    ps[:], w_sb[cc][:, dsl], xt[cc][:, sl],
                                    start=(cc == 0), stop=(cc == CC - 1),
                                )
                            nc.vector.tensor_scalar(
                                qtsb[:, sl], ps[:], b4[:, dc:dc + 1], None,
                                op0=ALU.add,
                            )
                        # rotate-half partners via PE pair-swap matmuls; a
                        # second pass so the swap of chunk 0 never stalls
                        # the PE behind chunk 0's bias-add on the DVE.
                        for st in range(4):
                            sl = slice(st * 512, (st + 1) * 512)
                            psw = PS.tile([128, 512], f32, tag="proj", name="psw")
                            nc.tensor.matmul(
                                psw[:], swp_t[:], qtsb[:, sl],
                                start=True, stop=True,
                            )
                            nc.vector.tensor_tensor(
                                dst[dc][:, sl], qtsb[:, sl], cos_t[:, sl],
                                op=ALU.mult,
                            )
                            nc.vector.tensor_tensor(
                                tt2[:, sl], psw[:], sin_t[:, sl], op=ALU.mult
                            )
                            nc.vector.tensor_tensor(
                                dst[dc][:, sl], dst[dc][:, sl], tt2[:, sl],
                                op=ALU.add,
                            )

                def emit_attn(dc):
                    # ---- attention for head pair dc ----
                    pr = dc
                    rscoll = NR.tile([65, NPP * QT2], f32, tag="rscoll", bufs=1, name="rscoll")
                    for q in range(NQ):
                        qsl = slice(q * QT2, (q + 1) * QT2)
                        pso = [
                            PS.tile([65, QT2], f32, tag="psoA", name="psoA", bufs=1),
                            PS.tile([65, QT2], f32, tag="psoB", name="psoB", bufs=1),
                        ]
                        for ks in range(SC):
                            ksl = slice(ks * 128, (ks + 1) * 128)
                            pss = PS.tile([128, 2 * QT2], f32, tag="big", name="pss")
                            for half in range(2):
                                rows = slice(64 * half, 64 * half + 64)
                                nc.tensor.matmul(
                                    pss[:, half * QT2:(half + 1) * QT2],
                                    kt[pr][rows, ksl],
                                    qt[pr][rows, qsl],
                                    start=True, stop=True,
                                )
                            ptile = PT.tile([128, 2 * QT2], bf16, tag="pt", name="ptile", bufs=3)
                            nc.scalar.activation(ptile[:], pss[:], AF.Exp, scale=0.125)
                            for half in range(2):
                                lh = 2 * pr + half
                                nc.tensor.matmul(
                                    pso[half][:],
                                    vaug[ks][:, 65 * lh:65 * lh + 65],
                                    ptile[:, half * QT2:(half + 1) * QT2],
                                    start=(ks == 0), stop=(ks == SC - 1),
                                )
                        # half 0 lands on partitions 0-63 directly (DVE);
                        # half 1 is cast by the DVE then partition-shifted
                        # to 64-127 by an SBUF-SBUF DMA (DVE lanes can't
                        # cross partitions, DMA can).
                        nc.vector.tensor_copy(aoh2[pr][0:64, qsl], pso[0][0:64, :])
                        tmpB = NR.tile([64, QT2], bf16, tag="tmpB", bufs=2)
                        nc.vector.tensor_copy(tmpB[:], pso[1][0:64, :])
                        nc.sync.dma_start(aoh2[pr][64:128, qsl], tmpB[:])
                        for half in range(2):
                            csl = slice((q * 2 + half) * QT2, (q * 2 + half + 1) * QT2)
                            nc.vector.tensor_copy(rscoll[64:65, csl], pso[half][64:65, :])
                    # batched denominators for this pair: one Ln + one Exp,
                    # then per-point broadcast and in-place normalize of aoh
                    nc.scalar.activation(rscoll[64:65, :], rscoll[64:65, :], AF.Ln)
                    nc.scalar.activation(
                        rscoll[64:65, :], rscoll[64:65, :], AF.Exp, scale=-1.0
                    )
                    for q in range(NQ):
                        qsl = slice(q * QT2, (q + 1) * QT2)
                        psb = PS.tile([128, QT2], f32, tag="psoA", name="psb", bufs=1)
                        for half in range(2):
                            csl = slice((q * 2 + half) * QT2, (q * 2 + half + 1) * QT2)
                            nc.tensor.matmul(
                                psb[64 * half:64 * half + 64, :],
                                ones65[64:65, 0:64],
                                rscoll[64:65, csl], start=True, stop=True,
                            )
                        recb = NR.tile([128, QT2], f32, tag="recb", bufs=1)
                        nc.vector.tensor_copy(recb[:], psb[:])
                        nc.vector.tensor_tensor(
                            aoh2[pr][:, qsl], aoh2[pr][:, qsl], recb[:],
                            op=ALU.mult,
                        )

                emit_qk(0)
                # ---- V projection first (attention needs all of it) ----
                wv_sb = [WV.tile([128, DG], bf16, tag="wv", name=f"wv_{_}") for _ in range(CC)]
                for i in range(CC):
                    nc.sync.dma_start(wv_sb[i][:], wv_d.ap()[i * 128:(i + 1) * 128, :])
                for sc in range(SC):
                    ssl = slice(sc * 128, (sc + 1) * 128)
                    ps = PS.tile([128, 512], f32, tag="proj", name="psv")
                    for cc in range(CC):
                        nc.tensor.matmul(
                            ps[:], xt[cc][:, ssl], wv_sb[cc][:],
                            start=(cc == 0), stop=False,
                        )
                    nc.tensor.matmul(
                        ps[:], ones[0:1, 0:128], bv_sb[:], start=False, stop=True,
                    )
                    va3 = vaug[sc][:].rearrange("p (h c) -> p h c", c=65)
                    ps3 = ps[:].rearrange("p (h c) -> p h c", c=64)
                    nc.vector.tensor_copy(va3[:, :, 0:64], ps3[:, :, :])
                    nc.vector.memset(va3[:, :, 64:65], 1.0)

                emit_attn(0)
                for dc in range(1, DC):
                    emit_qk(dc)
                    emit_attn(dc)

                # ---- output projection (K=128 per head pair) ----
                wo_sb = [P.tile([128, D], bf16, tag=f"wo{i}", name=f"wo{i}") for i in range(DC)]
                for i in range(DC):
                    nc.sync.dma_start(wo_sb[i][:], wo_d.ap()[i * 128:(i + 1) * 128, :])
                bo_sb = P.tile([1, D], bf16, tag="bo")
                nc.sync.dma_start(bo_sb[:], bo_d.ap()[:])
                for sc in range(SC):
                    ssl = slice(sc * 128, (sc + 1) * 128)
                    ps = PS.tile([128, 2 * QT2], f32, tag="big", name="pso3")
                    for nt in range(2):
                        nsl = slice(nt * 512, (nt + 1) * 512)
                        for j in range(DC):
                            nc.tensor.matmul(
                                ps[:, nsl], aoh2[j][:, ssl], wo_sb[j][:, nsl],
                                start=(j == 0), stop=False,
                            )
                        nc.tensor.matmul(
                            ps[:, nsl], ones[0:1, 0:128], bo_sb[0:1, nsl],
                            start=False, stop=True,
                        )
                    ob = T1.tile([128, D], f32, tag="qtsb", name="ob", bufs=2)
                    nc.vector.tensor_copy(ob[:], ps[:])
                    nc.sync.dma_start(out_d.ap()[ssl, :], ob[:])

    nc.compile()
    return nc


def _rope_tables(start):
    """Natural-layout rope tables: row p pairs (2f, 2f+1); sinTs carries
    the rotate-half sign (-1 on even rows, +1 on odd)."""
    inv_freq = (1.0 / (ROPE_BASE ** (np.arange(0, HDIM, 2, dtype=np.float64)
                                     / HDIM))).astype(np.float32)
    pos = np.arange(S, dtype=np.float32)
    rel = np.maximum(pos - start, 0.0)
    ang = inv_freq[:, None] * rel[None, :]          # [32, S]
    c64 = np.repeat(np.cos(ang), 2, axis=0)         # [64, S]
    s64 = np.repeat(np.sin(ang), 2, axis=0)
    sign = np.where(np.arange(HDIM) % 2 == 0, -1.0, 1.0).astype(np.float32)
    s64 = s64 * sign[:, None]
    cosT = np.concatenate([c64, c64], axis=0)       # [128, S]
    sinTs = np.concatenate([s64, s64], axis=0)
    return cosT, sinTs


def _swap_matrix():
    """SWP[k, m] = 1 iff m == k^1: psw = SWP^T-matmul gives psw[p]=q[p^1]."""
    swp = np.zeros((128, 128), dtype=np.float32)
    idx = np.arange(128)
    swp[idx, idx ^ 1] = 1.0
    return swp


def prepare_in_maps(inputs):
    import ml_dtypes

    bf16 = ml_dtypes.bfloat16
    x = np.asarray(inputs["x"])
    start = int(np.asarray(inputs["rope_start_index"]))

    jobs = {}
    for b in range(B):
        jobs[f"x{b}"] = _POOL.submit(
            lambda b=b: np.asarray(x[b], dtype=np.float32).astype(bf16))

    def prep_w(hg):
        csl = slice(hg * DG, (hg + 1) * DG)
        m = {}
        for name in ("q", "k", "v"):
            m["w" + name] = np.asarray(
                inputs["W" + name], dtype=np.float32)[:, csl].astype(bf16)
        m["wo"] = np.asarray(
            inputs["Wo"], dtype=np.float32)[csl, :].astype(bf16)
        for name in ("q", "k"):
            bvec = np.asarray(inputs["b" + name], dtype=np.float32)[csl]
            m["b" + name + "4"] = np.ascontiguousarray(
                bvec.reshape(4, 128).T).astype(np.float32)
        m["bv"] = np.asarray(inputs["bv"], dtype=np.float32)[None, csl].astype(bf16)
        bo = np.asarray(inputs["bo"], dtype=np.float32)
        m["bo"] = (bo if hg == 0 else np.zeros_like(bo))[None, :].astype(bf16)
        return m

    for hg in range(HG):
        jobs[f"w{hg}"] = _POOL.submit(prep_w, hg)
    jobs["tab"] = _POOL.submit(_rope_tables, start)

    cosT, sinTs = jobs["tab"].result()
    cosT = cosT.astype(bf16)
    sinTs = sinTs.astype(bf16)
    swp = _swap_matrix().astype(bf16)
    xbs = [jobs[f"x{b}"].result() for b in range(B)]
    per_hg = [jobs[f"w{hg}"].result() for hg in range(HG)]

    _nw = (int(os.environ.get("BUILD_REPEAT", "1"))
           + 100 * int(os.environ.get("BUILD_NONCE", "0")))
    nonce = np.zeros((1, _nw), np.float32)
    in_maps = []
    for c in range(NCORES):
        b, hg = c // HG, c % HG
        m = per_hg[hg]
        in_maps.append({
            "nonce": nonce,
            "x": xbs[b],
            "wq": m["wq"], "wk": m["wk"], "wv": m["wv"], "wo": m["wo"],
            "cosT": cosT, "sinTs": sinTs, "swp": swp,
            "bq4": m["bq4"], "bk4": m["bk4"],
            "bv": m["bv"], "bo": m["bo"],
        })
    return in_maps


def kernel(**inputs):
    from concourse.bass_utils import run_bass_kernel_spmd

    if "nc" not in _CACHE:
        _CACHE["nc"] = _build_program()
    nc = _CACHE["nc"]

    in_maps = prepare_in_maps(inputs)
    res = run_bass_kernel_spmd(nc, in_maps, core_ids=list(range(NCORES)))
    out = np.empty((B, S, D), dtype=np.float32)

    def assemble(b):
        np.add(res.results[HG * b]["out"], res.results[HG * b + 1]["out"],
               out=out[b])
    list(_POOL.map(assemble, range(B)))
    return out
